# revision 30
# baseline (speedup 1.0000x reference)
"""Trainium2 Bass kernel for nn_DiscAdvLossForTarget_min.

Math (per batch row, x = logits[0:1000], e = extra logit x[1000]):
    prob_i = softmax(x)_i                  = exp(x_i - e) / sum_j exp(x_j - e)
    log pc_i = log sigmoid(e - x_i)        = -log1p(exp(x_i - e))
    loss = -(1/B) * sum_b sum_i prob_i * log(pc_i)
         = +(1/B) * sum_b U_b / S_b
    where a_i = exp(x_i - e), U_b = sum_i a_i * log1p(a_i), S_b = sum_i a_i.

Device mapping (per core, data-parallel over batch, 8192 rows per core).
Work is emitted in supertiles of g row-blocks (128 rows each, one DMA per
supertile); the supertile plan ramps 2,2,4,8,...,8,4,2,2 so the pipeline
fills and drains with small dependency chains. Within a supertile the
S-row-sum work is split between the two row-sum-capable engines so
neither is the bottleneck:
  mode A (first ka blocks): ACT a = Exp(x + bias(-e)), accum_out -> S col
  mode D (rest):  one batched ACT Exp -> t; DVE tensor_scalar
      a = t * exp(-e) with accum_out -> S col (1x reduce path)
  then one batched ACT Ln(a + 1) -> w (same table set as Exp), and a DVE
  scalar_tensor_tensor (a * 1) * w with accum_out -> U col per block.
Intermediates a/t/w are bf16 (halves SBUF; accumulators stay fp32).
Host: loss = (1/B) * sum over rows/cores of U/S.
"""

import numpy as np

import bass_rust as _bass_rust
import concourse.bacc as bacc
import concourse.bass as bass
import concourse.tile as tile
from concourse import bass_utils, mybir
from concourse.hw_specs import get_activation_tables

N_CORES = 8
B_FULL = 65536
C1 = 1001
C = 1000
P = 128
B_SHARD = B_FULL // N_CORES  # 8192
N_BLOCKS = B_SHARD // P  # 64
G_MAX = 8

# supertile sizes (sum = N_BLOCKS); small at the ends to shorten pipeline
# fill and drain. ka = blocks whose S-reduce rides the ACT Exp accumulator.
PLAN = [2, 2, 4] + [8] * 7
assert sum(PLAN) == N_BLOCKS
KA_OF = {2: 1, 4: 2, 8: 4}


class _PinnedBacc(bacc.Bacc):
    """Bacc whose activation-table chooser only sees sets containing every
    activation function this kernel uses, so Exp and Ln resolve to one
    resident set (natural_log_exp_and_others) instead of thrashing
    ACT_TABLE_LOADs between per-function sets."""

    def insert_act_table_loads(self):
        used = {
            i.func
            for b in self.main_func.blocks
            for i in b.instructions
            if isinstance(i, mybir.InstActivation)
        }
        if not used:
            return
        tables = [
            (name, fns if used <= fns else set())
            for name, fns in get_activation_tables(self.m.arch).items()
        ]
        _bass_rust.insert_act_table_loads(self, tables)


_nc_cache = None


def _build() -> bass.Bass:
    global _nc_cache
    if _nc_cache is not None:
        return _nc_cache

    nc = _PinnedBacc("TRN2", debug=False)
    x = nc.dram_tensor("x", [B_SHARD, C1], mybir.dt.float32, kind="ExternalInput").ap()
    u_out = nc.dram_tensor(
        "u_out", [P, N_BLOCKS], mybir.dt.float32, kind="ExternalOutput"
    ).ap()
    s_out = nc.dram_tensor(
        "s_out", [P, N_BLOCKS], mybir.dt.float32, kind="ExternalOutput"
    ).ap()

    # Shard row handled by (partition p, block n): row = p*N_BLOCKS + n, so a
    # run of consecutive blocks is contiguous DRAM per partition.
    x_r = x.rearrange("(p n) m -> p n m", p=P, n=N_BLOCKS)

    with tile.TileContext(nc) as tc:
        with (
            tc.tile_pool(name="xin", bufs=2) as xin,
            tc.tile_pool(name="mid", bufs=2) as mid,
            tc.tile_pool(name="small", bufs=3) as small,
            tc.tile_pool(name="accp", bufs=1) as accp,
        ):
            U = accp.tile([P, N_BLOCKS], mybir.dt.float32)
            S = accp.tile([P, N_BLOCKS], mybir.dt.float32)
            n0 = 0  # first block of the current supertile
            for g in PLAN:
                ka = KA_OF[g]
                kd = g - ka
                xt = xin.tile([P, G_MAX, C1], mybir.dt.float32, tag="xt")
                nc.sync.dma_start(out=xt[:, 0:g, :], in_=x_r[:, n0 : n0 + g, :])

                aa = mid.tile([P, G_MAX, C], mybir.dt.bfloat16, tag="aa")

                # mode A: Exp with per-partition bias(-e) + accumulator row-sum
                neg_e = small.tile([P, G_MAX], mybir.dt.float32, tag="neg_e")
                nc.vector.tensor_scalar_mul(neg_e[:, 0:ka], xt[:, 0:ka, C], -1.0)
                for i in range(ka):
                    col = n0 + i
                    nc.scalar.activation(
                        out=aa[:, i, :],
                        in_=xt[:, i, 0:C],
                        func=mybir.ActivationFunctionType.Exp,
                        bias=neg_e[:, i : i + 1],
                        scale=1.0,
                        accum_out=S[:, col : col + 1],
                    )

                # mode D: batched Exp (incl. e columns), then DVE scale+reduce
                tt = mid.tile([P, G_MAX // 2, C1], mybir.dt.bfloat16, tag="tt")
                nc.scalar.activation(
                    out=tt[:, 0:kd, :].rearrange("p g c -> p (g c)"),
                    in_=xt[:, ka:g, :].rearrange("p g c -> p (g c)"),
                    func=mybir.ActivationFunctionType.Exp,
                )
                cc = small.tile([P, G_MAX], mybir.dt.float32, tag="cc")
                nc.vector.reciprocal(cc[:, 0:kd], tt[:, 0:kd, C])
                for j in range(kd):
                    # plain 4x-mode bf16 scale (no accumulator)
                    nc.vector.tensor_scalar_mul(
                        aa[:, ka + j, :], tt[:, j, 0:C], cc[:, j : j + 1]
                    )
                if kd:
                    # direct multi-block row-sum, no accumulator register
                    nc.vector.reduce_sum(
                        S[:, n0 + ka : n0 + g],
                        aa[:, ka:g, :],
                        axis=mybir.AxisListType.X,
                    )

                ww = mid.tile([P, G_MAX, C], mybir.dt.bfloat16, tag="ww")
                nc.scalar.activation(
                    out=ww[:, 0:g, :].rearrange("p g c -> p (g c)"),
                    in_=aa[:, 0:g, :].rearrange("p g c -> p (g c)"),
                    func=mybir.ActivationFunctionType.Ln,
                    bias=1.0,
                    scale=1.0,
                )

                # product in one 2x-mode bf16 pass, then one direct
                # multi-block row-sum (no accumulator registers involved)
                uu = mid.tile([P, G_MAX, C], mybir.dt.bfloat16, tag="uu")
                nc.vector.tensor_mul(
                    uu[:, 0:g, :].rearrange("p g c -> p (g c)"),
                    aa[:, 0:g, :].rearrange("p g c -> p (g c)"),
                    ww[:, 0:g, :].rearrange("p g c -> p (g c)"),
                )
                nc.vector.reduce_sum(
                    U[:, n0 : n0 + g],
                    uu[:, 0:g, :],
                    axis=mybir.AxisListType.X,
                )
                n0 += g

            nc.sync.dma_start(out=u_out, in_=U)
            nc.sync.dma_start(out=s_out, in_=S)

    nc.finalize()  # runs Bacc passes (wait splitting, reg alloc, ...)
    _nc_cache = nc
    return nc


LAST_RESULTS = None


def kernel(input: np.ndarray, target: np.ndarray | None = None, _trace: bool = False, **_unused) -> np.ndarray:
    global LAST_RESULTS
    input = np.ascontiguousarray(np.asarray(input, dtype=np.float32))
    assert input.shape == (B_FULL, C1), input.shape

    nc = _build()
    in_maps = [
        {"x": input[i * B_SHARD : (i + 1) * B_SHARD]} for i in range(N_CORES)
    ]
    res = bass_utils.run_bass_kernel_spmd(
        nc, in_maps, core_ids=list(range(N_CORES)), trace=_trace
    )
    LAST_RESULTS = res
    total = np.float64(0.0)
    for r in res.results:
        u = np.asarray(r["u_out"], dtype=np.float64)
        s = np.asarray(r["s_out"], dtype=np.float64)
        total += (u / s).sum()
    # w = log1p(a) = -log(pc) already carries the loss's minus sign.
    loss = total / B_FULL
    return np.float32(loss)


# revision 31
# speedup vs baseline: 1.1616x; 1.1616x over previous
"""Trainium2 Bass kernel for nn_DiscAdvLossForTarget_min.

Math (per batch row, x = logits[0:1000], e = extra logit x[1000]):
    prob_i = softmax(x)_i                  = exp(x_i - e) / sum_j exp(x_j - e)
    log pc_i = log sigmoid(e - x_i)        = -log1p(exp(x_i - e))
    loss = -(1/B) * sum_b sum_i prob_i * log(pc_i)
         = +(1/B) * sum_b U_b / S_b
    where a_i = exp(x_i - e), U_b = sum_i a_i * log1p(a_i), S_b = sum_i a_i.

Device mapping (per core, data-parallel over batch, 8192 rows per core).
Work is emitted in supertiles of g row-blocks (128 rows each, one DMA per
supertile); the supertile plan ramps 2,2,4,8,...,8,4,2,2 so the pipeline
fills and drains with small dependency chains. Within a supertile the
S-row-sum work is split between the two row-sum-capable engines so
neither is the bottleneck:
  mode A (first ka blocks): ACT a = Exp(x + bias(-e)), accum_out -> S col
  mode D (rest):  one batched ACT Exp -> t; DVE tensor_scalar
      a = t * exp(-e) with accum_out -> S col (1x reduce path)
  then one batched ACT Ln(a + 1) -> w (same table set as Exp), and a DVE
  scalar_tensor_tensor (a * 1) * w with accum_out -> U col per block.
Intermediates a/t/w are bf16 (halves SBUF; accumulators stay fp32).
Host: loss = (1/B) * sum over rows/cores of U/S.
"""

import numpy as np

import bass_rust as _bass_rust
import concourse.bacc as bacc
import concourse.bass as bass
import concourse.tile as tile
from concourse import bass_utils, mybir
from concourse.hw_specs import get_activation_tables

N_CORES = 8
B_FULL = 65536
C1 = 1001
C = 1000
P = 128
B_SHARD = B_FULL // N_CORES  # 8192
N_BLOCKS = B_SHARD // P  # 64
G_MAX = 8

# supertile sizes (sum = N_BLOCKS); small at the ends to shorten pipeline
# fill and drain. ka = blocks whose S-reduce rides the ACT Exp accumulator.
PLAN = [2, 2, 4] + [8] * 7
assert sum(PLAN) == N_BLOCKS
KA_OF = {2: 1, 4: 2, 8: 4}


class _PinnedBacc(bacc.Bacc):
    """Bacc whose activation-table chooser only sees sets containing every
    activation function this kernel uses, so Exp and Ln resolve to one
    resident set (natural_log_exp_and_others) instead of thrashing
    ACT_TABLE_LOADs between per-function sets."""

    def insert_act_table_loads(self):
        used = {
            i.func
            for b in self.main_func.blocks
            for i in b.instructions
            if isinstance(i, mybir.InstActivation)
        }
        if not used:
            return
        tables = [
            (name, fns if used <= fns else set())
            for name, fns in get_activation_tables(self.m.arch).items()
        ]
        _bass_rust.insert_act_table_loads(self, tables)


_nc_cache = None


def _build() -> bass.Bass:
    global _nc_cache
    if _nc_cache is not None:
        return _nc_cache

    nc = _PinnedBacc("TRN2", debug=False)
    x = nc.dram_tensor("x", [B_SHARD, C1], mybir.dt.float32, kind="ExternalInput").ap()
    u_out = nc.dram_tensor(
        "u_out", [P, N_BLOCKS], mybir.dt.float32, kind="ExternalOutput"
    ).ap()
    s_out = nc.dram_tensor(
        "s_out", [P, N_BLOCKS], mybir.dt.float32, kind="ExternalOutput"
    ).ap()

    # Shard row handled by (partition p, block n): row = p*N_BLOCKS + n, so a
    # run of consecutive blocks is contiguous DRAM per partition.
    x_r = x.rearrange("(p n) m -> p n m", p=P, n=N_BLOCKS)

    with tile.TileContext(nc) as tc:
        with (
            tc.tile_pool(name="xin", bufs=3) as xin,
            tc.tile_pool(name="mid", bufs=2) as mid,
            tc.tile_pool(name="small", bufs=3) as small,
            tc.tile_pool(name="accp", bufs=1) as accp,
        ):
            U = accp.tile([P, N_BLOCKS], mybir.dt.float32)
            S = accp.tile([P, N_BLOCKS], mybir.dt.float32)
            n0 = 0  # first block of the current supertile
            for g in PLAN:
                ka = KA_OF[g]
                kd = g - ka
                xt = xin.tile([P, G_MAX, C1], mybir.dt.float32, tag="xt")
                nc.sync.dma_start(out=xt[:, 0:g, :], in_=x_r[:, n0 : n0 + g, :])

                aa = mid.tile([P, G_MAX, C], mybir.dt.bfloat16, tag="aa")

                # mode A: Exp with per-partition bias(-e) + accumulator row-sum
                neg_e = small.tile([P, G_MAX], mybir.dt.float32, tag="neg_e")
                nc.vector.tensor_scalar_mul(neg_e[:, 0:ka], xt[:, 0:ka, C], -1.0)
                for i in range(ka):
                    col = n0 + i
                    nc.scalar.activation(
                        out=aa[:, i, :],
                        in_=xt[:, i, 0:C],
                        func=mybir.ActivationFunctionType.Exp,
                        bias=neg_e[:, i : i + 1],
                        scale=1.0,
                        accum_out=S[:, col : col + 1],
                    )

                # mode D: batched Exp (incl. e columns), then DVE scale+reduce
                tt = mid.tile([P, G_MAX, C1], mybir.dt.bfloat16, tag="tt")
                nc.scalar.activation(
                    out=tt[:, 0:kd, :].rearrange("p g c -> p (g c)"),
                    in_=xt[:, ka:g, :].rearrange("p g c -> p (g c)"),
                    func=mybir.ActivationFunctionType.Exp,
                )
                cc = small.tile([P, G_MAX], mybir.dt.float32, tag="cc")
                nc.vector.reciprocal(cc[:, 0:kd], tt[:, 0:kd, C])
                for j in range(kd):
                    # plain 4x-mode bf16 scale (no accumulator)
                    nc.vector.tensor_scalar_mul(
                        aa[:, ka + j, :], tt[:, j, 0:C], cc[:, j : j + 1]
                    )
                if kd:
                    # direct multi-block row-sum, no accumulator register
                    nc.vector.reduce_sum(
                        S[:, n0 + ka : n0 + g],
                        aa[:, ka:g, :],
                        axis=mybir.AxisListType.X,
                    )

                ww = mid.tile([P, G_MAX, C], mybir.dt.bfloat16, tag="ww")
                nc.scalar.activation(
                    out=ww[:, 0:g, :].rearrange("p g c -> p (g c)"),
                    in_=aa[:, 0:g, :].rearrange("p g c -> p (g c)"),
                    func=mybir.ActivationFunctionType.Ln,
                    bias=1.0,
                    scale=1.0,
                )

                for i in range(g):
                    col = n0 + i
                    scr = mid.tile([P, C], mybir.dt.bfloat16, tag="scr")
                    nc.vector.scalar_tensor_tensor(
                        out=scr,
                        in0=aa[:, i, :],
                        scalar=1.0,
                        in1=ww[:, i, :],
                        op0=mybir.AluOpType.mult,
                        op1=mybir.AluOpType.mult,
                        accum_out=U[:, col : col + 1],
                    )
                n0 += g

            nc.sync.dma_start(out=u_out, in_=U)
            nc.sync.dma_start(out=s_out, in_=S)

    nc.finalize()  # runs Bacc passes (wait splitting, reg alloc, ...)
    _nc_cache = nc
    return nc


LAST_RESULTS = None


def kernel(input: np.ndarray, target: np.ndarray | None = None, _trace: bool = False, **_unused) -> np.ndarray:
    global LAST_RESULTS
    input = np.ascontiguousarray(np.asarray(input, dtype=np.float32))
    assert input.shape == (B_FULL, C1), input.shape

    nc = _build()
    in_maps = [
        {"x": input[i * B_SHARD : (i + 1) * B_SHARD]} for i in range(N_CORES)
    ]
    res = bass_utils.run_bass_kernel_spmd(
        nc, in_maps, core_ids=list(range(N_CORES)), trace=_trace
    )
    LAST_RESULTS = res
    total = np.float64(0.0)
    for r in res.results:
        u = np.asarray(r["u_out"], dtype=np.float64)
        s = np.asarray(r["s_out"], dtype=np.float64)
        total += (u / s).sum()
    # w = log1p(a) = -log(pc) already carries the loss's minus sign.
    loss = total / B_FULL
    return np.float32(loss)
